# revision 1
# baseline (speedup 1.0000x reference)
"""Trainium2 Bass kernel for nn_LinearCondensed.

Computes out[b, o] = sum_k weight[o, k] * x[b, indx_seqs[o, k]] + bias[o]
with B=2048, IN_F=OUT_F=4096, FAN_IN=32.

Strategy: the gather has no fast on-chip primitive (any materialized gather
moves 32x the data of x itself), so we densify the sparse weight matrix on
the host -- W'[o, i] = sum_{k: indx_seqs[o,k]==i} weight[o, k] -- and run a
dense fp32r matmul out = x @ W'^T + bias on the PE array, which streams at
1 cycle/row (bf16 speed) for moving dims >= 256. OUT_F is sharded 8 ways
across cores (512 columns each), x is replicated, bias is folded in as a
K=1 matmul against a ones vector. Host also pre-tiles both operands into
the exact SBUF layouts so every DMA is a large contiguous copy.
"""

import os
import sys
import types

import numpy as np

import concourse.bacc as bacc
import concourse.mybir as mybir
import concourse.tile as tile
from concourse.bass_utils import run_bass_kernel_spmd

B, IN_F, OUT_F, FAN_IN = 2048, 4096, 4096, 32
NCORES = 8
OSH = OUT_F // NCORES          # 512 output features per core
P = 128                        # partitions
BT = B // P                    # 16 batch tiles
KT = IN_F // P                 # 32 contraction tiles
N = OSH                        # 512 moving columns (max for fp32)

f32 = mybir.dt.float32
f32r = mybir.dt.float32r

_cache = {}


def _enable_ntff_hook():
    """Register the ctypes NTFF profile hook (the image's antenv lacks
    axon_hooks); lets trace=True produce a neuron-profile under axon."""
    try:
        from antenv.axon_hooks import get_axon_ntff_profile_hook  # noqa: F401
        return
    except ImportError:
        pass
    try:
        import antenv
        from trn_agent_boot.trn_boot import _ntff_profile_via_ctypes

        mod = types.ModuleType("antenv.axon_hooks")
        holder = [None]
        mod.set_axon_ntff_profile_hook = lambda h: holder.__setitem__(0, h)
        mod.get_axon_ntff_profile_hook = lambda: holder[0]
        antenv.axon_hooks = mod
        sys.modules["antenv.axon_hooks"] = mod
        mod.set_axon_ntff_profile_hook(
            _ntff_profile_via_ctypes("/opt/axon/libaxon_pjrt.so"))
        import concourse.bass_utils as bu
        bu.upload_artifacts = lambda tmpdir: str(tmpdir)
    except Exception:
        pass


def _build():
    nc = bacc.Bacc()
    # xt[t] is the (128p=i-within-ktile, KT*128=b columns... see layout below)
    # Layouts (host-pretiled, all contiguous):
    #   XT[t, p, a, c] = x[t*128 + c, a*128 + p]   -> per b-tile t: [128, KT*128]
    #   WT[p, a, n]    = W'[o0 + n, a*128 + p]     -> [128, KT*512]
    XT = nc.declare_dram_parameter("XT", [BT, P, KT * P], f32r, isOutput=False)
    WT = nc.declare_dram_parameter("WT", [KT, P, N], f32r, isOutput=False)
    BIAS = nc.declare_dram_parameter("BIAS", [P, N], f32, isOutput=False)
    OUT = nc.declare_dram_parameter("OUT", [B, N], f32, isOutput=True)

    XTv = XT.ap().rearrange("t p (a c) -> t p a c", a=KT)

    with tile.TileContext(nc) as tc:
        with (
            tc.tile_pool(name="wpool", bufs=1) as wpool,
            tc.tile_pool(name="xpool", bufs=4) as xpool,
            tc.tile_pool(name="cpool", bufs=1) as cpool,
            tc.tile_pool(name="opool", bufs=3) as opool,
            tc.tile_pool(name="psum", bufs=4, space="PSUM") as psum,
        ):
            # All input loads ride the single sync HWDGE FIFO in a deliberate
            # order: x0, x1 at full bandwidth (PE can start at ~6us), then
            # the 32 weight k-tiles (which pace b-tile 0), then x2+ arrive
            # just in time. Output stores use the scalar HWDGE queue so they
            # never block input loads.
            xtiles = {}

            def load_x(t):
                xs = xpool.tile([P, KT, P], f32r, tag="xs")
                nc.sync.dma_start(xs[:], XTv[t])
                xtiles[t] = xs

            load_x(0)
            load_x(1)
            # weights in 8 groups of 4 k-tiles (1MB per DMA): few enough
            # triggers (~0.65us each on the issuing engine) to not serialize
            # the start, fine-grained enough to pace b-tile 0.
            WG = 4
            wgroups = []
            brow = ones = None
            for g in range(KT // WG):
                w = wpool.tile([P, WG, N], f32r, tag=f"w{g}")
                nc.sync.dma_start(
                    w[:], WT.ap().rearrange("(g j) p n -> g p j n", j=WG)[g])
                wgroups.append(w)
                if g == 1:
                    brow = cpool.tile([P, N], f32)
                    nc.sync.dma_start(brow[:], BIAS[:])
            wtiles = [wgroups[a // WG][:, a % WG, :] for a in range(KT)]

            # bias folded into the PSUM drain: osb = acc + bias (bias row
            # pre-replicated across partitions on host), saving 16 K=1 bias
            # matmuls on the PE.
            def finish_tile(t, acc):
                osb = opool.tile([P, N], f32, tag="osb")
                nc.vector.tensor_tensor(osb[:], acc[:], brow[:], mybir.AluOpType.add)
                nc.scalar.dma_start(OUT.ap()[t * P:(t + 1) * P, :], osb[:])

            # Phase 1: b-tiles 0-1 in k-outer order so the PE consumes each
            # weight group as it lands instead of idling through the 8MB
            # weight stream.
            G = 2
            accs = [psum.tile([P, N], f32, name=f"acc{t}", tag="acc")
                    for t in range(G)]
            for a in range(KT):
                for t in range(G):
                    nc.tensor.matmul(
                        accs[t][:], xtiles[t][:, a, :], wtiles[a][:],
                        start=(a == 0), stop=(a == KT - 1),
                    )
            for t in range(G):
                finish_tile(t, accs[t])

            # Phase 2: remaining b-tiles, k-inner, x streamed just in time.
            for t in range(G, BT):
                load_x(t)
                xsb = xtiles[t]
                acc = psum.tile([P, N], f32, tag="acc")
                for a in range(KT):
                    nc.tensor.matmul(
                        acc[:],
                        xsb[:, a, :],      # lhsT: [K=128 (i), M=128 (b)]
                        wtiles[a][:],      # rhs:  [K=128 (i), N=512 (o)]
                        start=(a == 0),
                        stop=(a == KT - 1),
                    )
                finish_tile(t, acc)

    nc.compile()
    return nc


def kernel(x, weight, bias, indx_seqs):
    x = np.asarray(x, dtype=np.float32)
    weight = np.asarray(weight, dtype=np.float32)
    bias = np.asarray(bias, dtype=np.float32)
    indx_seqs = np.asarray(indx_seqs)

    if "nc" not in _cache:
        _cache["nc"] = _build()
    nc = _cache["nc"]

    # Densify sparse weights: W'[o, i] += weight[o, k] at i = indx_seqs[o, k]
    wd = np.zeros((OUT_F, IN_F), dtype=np.float32)
    np.add.at(wd, (np.arange(OUT_F)[:, None], indx_seqs), weight)

    # Host pre-tiling into SBUF-friendly layouts.
    # XT[t, p, a, c] = x[t*128+c, a*128+p]
    xt = np.ascontiguousarray(
        x.reshape(BT, P, KT, P).transpose(0, 3, 2, 1)
    ).reshape(BT, P, KT * P)
    in_maps = []
    for c in range(NCORES):
        wshard = wd[c * OSH:(c + 1) * OSH]            # (512, 4096)
        # WT[a, p, n] = W'[o0+n, a*128+p]
        wt = np.ascontiguousarray(
            wshard.reshape(OSH, KT, P).transpose(1, 2, 0))
        in_maps.append({
            "XT": xt,
            "WT": wt,
            "BIAS": np.ascontiguousarray(np.broadcast_to(bias[c * OSH:(c + 1) * OSH], (P, N))),
        })

    trace = bool(int(os.environ.get("BASSK_TRACE", "0"))) or bool(
        os.environ.get("BASS_TRACE"))
    if trace:
        _enable_ntff_hook()
    res = run_bass_kernel_spmd(
        nc, in_maps, list(range(NCORES)), trace=trace,
        trace_cores=list(range(NCORES)) if trace else None,
    )
    _cache["last_results"] = res

    out = np.concatenate([res.results[c]["OUT"] for c in range(NCORES)], axis=1)
    return out



# revision 6
# speedup vs baseline: 1.1599x; 1.1599x over previous
"""Trainium2 Bass kernel for nn_LinearCondensed.

Computes out[b, o] = sum_k weight[o, k] * x[b, indx_seqs[o, k]] + bias[o]
with B=2048, IN_F=OUT_F=4096, FAN_IN=32.

Strategy: the gather has no fast on-chip primitive (any materialized gather
moves 32x the data of x itself), so we densify the sparse weight matrix on
the host -- W'[o, i] = sum_{k: indx_seqs[o,k]==i} weight[o, k] -- and run a
dense matmul out = x @ W'^T + bias on the PE array. OUT_F is sharded 8 ways
across cores (512 columns each), x is replicated.

v2 over the fp32r baseline:
  * Operands in bf16: halves HBM traffic (44MB -> ~24MB per core) so the
    kernel is PE-bound, and enables Fast Weight Load so the per-matmul
    LDWEIGHTS fully hides under the 512-column stream.
  * DMA order puts the first weight groups before the bulk of x, and the
    first x tile is split in half, so the PE starts at ~6us (was 25us).
  * Phase 1 is a wavefront: partial k-ranges of b-tiles 0-2 are issued as
    soon as their (x tile, weight group) pair has landed, using 3 live PSUM
    banks, so the PE is never weight-paced for long during the 4MB weight
    stream.
"""

import os
import sys
import types

import ml_dtypes
import numpy as np

import concourse.bacc as bacc
import concourse.mybir as mybir
import concourse.tile as tile
from concourse.bass_utils import run_bass_kernel_spmd

B, IN_F, OUT_F, FAN_IN = 2048, 4096, 4096, 32
NCORES = 8
OSH = OUT_F // NCORES          # 512 output features per core
P = 128                        # partitions
BT = B // P                    # 16 batch tiles
KT = IN_F // P                 # 32 contraction tiles
N = OSH                        # 512 moving columns
WG = 4                         # k-tiles per weight DMA group
NG = KT // WG                  # 8 weight groups

f32 = mybir.dt.float32
bf16 = mybir.dt.bfloat16

_cache = {}


def _enable_ntff_hook():
    """Register the ctypes NTFF profile hook (the image's antenv lacks
    axon_hooks); lets trace=True produce a neuron-profile under axon."""
    try:
        from antenv.axon_hooks import get_axon_ntff_profile_hook  # noqa: F401
        return
    except ImportError:
        pass
    try:
        import antenv
        from trn_agent_boot.trn_boot import _ntff_profile_via_ctypes

        mod = types.ModuleType("antenv.axon_hooks")
        holder = [None]
        mod.set_axon_ntff_profile_hook = lambda h: holder.__setitem__(0, h)
        mod.get_axon_ntff_profile_hook = lambda: holder[0]
        antenv.axon_hooks = mod
        sys.modules["antenv.axon_hooks"] = mod
        mod.set_axon_ntff_profile_hook(
            _ntff_profile_via_ctypes("/opt/axon/libaxon_pjrt.so"))
        import concourse.bass_utils as bu
        bu.upload_artifacts = lambda tmpdir: str(tmpdir)
    except Exception:
        pass


def _build():
    nc = bacc.Bacc()
    # Layouts (host-pretiled, all contiguous):
    #   XT[t, p, a, c]    = x[t*128 + c, a*128 + p]    -> per b-tile t: [128, KT*128]
    #   WT[g, p, j, n]    = W'[o0 + n, (g*4+j)*128+p]  -> per group g: [128, 4*512]
    #   BIAS[p, n]        = bias[o0 + n]               (replicated across p)
    XT = nc.declare_dram_parameter("XT", [BT, P, KT * P], bf16, isOutput=False)
    WT = nc.declare_dram_parameter("WT", [NG, P, WG * N], bf16, isOutput=False)
    BIAS = nc.declare_dram_parameter("BIAS", [P, N], f32, isOutput=False)
    OUT = nc.declare_dram_parameter("OUT", [B, N], f32, isOutput=True)

    XTv = XT.ap().rearrange("t p (a c) -> t p a c", a=KT)
    WTv = WT.ap().rearrange("g p (j n) -> g p j n", j=WG)

    with tile.TileContext(nc) as tc:
        with (
            tc.tile_pool(name="wpool", bufs=1) as wpool,
            tc.tile_pool(name="xpool", bufs=1) as xpool,
            tc.tile_pool(name="cpool", bufs=1) as cpool,
            tc.tile_pool(name="opool", bufs=4) as opool,
            tc.tile_pool(name="psum", bufs=5, space="PSUM") as psum,
        ):
            # Everything is statically allocated (16MB x + 4MB W bf16 fits
            # SBUF) so no ring-reuse dependency can ever stall a DMA.
            xtiles = {}
            wgroups = [None] * NG
            dma_q = []  # (kind, idx) issue order on the sync HWDGE queue

            def load_wg(g):
                w = wpool.tile([P, WG, N], bf16, tag=f"w{g}")
                nc.sync.dma_start(w[:], WTv[g])
                wgroups[g] = w

            def load_x(t):
                xs = xpool.tile([P, KT, P], bf16, tag=f"xs{t}")
                nc.sync.dma_start(xs[:], XTv[t])
                xtiles[t] = xs

            # t=0's x rides in two halves so the PE can start after ~1MB.
            xs0a = xpool.tile([P, KT // 2, P], bf16, tag="xs0a")
            xs0b = xpool.tile([P, KT // 2, P], bf16, tag="xs0b")

            def xsl(t, a):
                if t == 0:
                    return (xs0a if a < KT // 2 else xs0b)[:, a % (KT // 2), :]
                return xtiles[t][:, a, :]

            # DMA issue order (single sync FIFO => arrival order):
            nc.sync.dma_start(xs0a[:], XTv[0][:, :KT // 2, :])
            load_wg(0)
            load_wg(1)
            load_wg(2)
            load_wg(3)
            load_x(1)
            nc.sync.dma_start(xs0b[:], XTv[0][:, KT // 2:, :])
            brow = cpool.tile([P, N], f32)
            nc.sync.dma_start(brow[:], BIAS[:])
            load_wg(4)
            load_wg(5)
            load_x(2)
            load_wg(6)
            load_wg(7)
            for t in range(3, BT):
                load_x(t)

            accs = {}

            def mm(t, a0, a1):
                for a in range(a0, a1):
                    nc.tensor.matmul(
                        accs[t][:], xsl(t, a), wgroups[a // WG][:, a % WG, :],
                        start=(a == 0), stop=(a == KT - 1),
                    )

            def finish(t):
                osb = opool.tile([P, N], f32, tag="osb")
                nc.vector.tensor_tensor(osb[:], accs[t][:], brow[:],
                                        mybir.AluOpType.add)
                nc.scalar.dma_start(OUT.ap()[t * P:(t + 1) * P, :], osb[:])

            # Phase 1 wavefront: follow the weight-group/x arrivals.
            for t in range(3):
                accs[t] = psum.tile([P, N], f32, name=f"acc{t}", tag="acc")
            mm(0, 0, 4)
            mm(0, 4, 8)
            mm(0, 8, 12)
            mm(0, 12, 16)
            mm(1, 0, 16)
            mm(0, 16, 24)
            mm(1, 16, 24)
            mm(2, 0, 16)
            mm(0, 24, 32)
            finish(0)
            mm(1, 24, 32)
            finish(1)
            mm(2, 16, 32)
            finish(2)

            # Phase 2: remaining b-tiles, k-inner, x already streaming.
            for t in range(3, BT):
                accs[t] = psum.tile([P, N], f32, name=f"accp{t}", tag="acc")
                mm(t, 0, KT)
                finish(t)

    nc.compile()
    return nc


def kernel(x, weight, bias, indx_seqs):
    x = np.asarray(x, dtype=np.float32)
    weight = np.asarray(weight, dtype=np.float32)
    bias = np.asarray(bias, dtype=np.float32)
    indx_seqs = np.asarray(indx_seqs)

    if "nc" not in _cache:
        _cache["nc"] = _build()
    nc = _cache["nc"]

    # Densify sparse weights: W'[o, i] += weight[o, k] at i = indx_seqs[o, k]
    wd = np.zeros((OUT_F, IN_F), dtype=np.float32)
    np.add.at(wd, (np.arange(OUT_F)[:, None], indx_seqs), weight)

    # Host pre-tiling into SBUF-friendly layouts (bf16).
    # XT[t, p, a, c] = x[t*128+c, a*128+p]
    xt = np.ascontiguousarray(
        x.reshape(BT, P, KT, P).transpose(0, 3, 2, 1)
    ).reshape(BT, P, KT * P).astype(ml_dtypes.bfloat16)
    in_maps = []
    for c in range(NCORES):
        wshard = wd[c * OSH:(c + 1) * OSH]            # (512, 4096)
        # WT[g, p, j, n] = W'[o0+n, (g*4+j)*128+p]
        wt = np.ascontiguousarray(
            wshard.reshape(OSH, NG, WG, P).transpose(1, 3, 2, 0)
        ).reshape(NG, P, WG * N).astype(ml_dtypes.bfloat16)
        in_maps.append({
            "XT": xt,
            "WT": wt,
            "BIAS": np.ascontiguousarray(
                np.broadcast_to(bias[c * OSH:(c + 1) * OSH], (P, N))),
        })

    trace = bool(int(os.environ.get("BASSK_TRACE", "0"))) or bool(
        os.environ.get("BASS_TRACE"))
    if trace:
        _enable_ntff_hook()
    res = run_bass_kernel_spmd(
        nc, in_maps, list(range(NCORES)), trace=trace,
        trace_cores=list(range(NCORES)) if trace else None,
    )
    _cache["last_results"] = res

    out = np.concatenate([res.results[c]["OUT"] for c in range(NCORES)], axis=1)
    return out
